# revision 41
# baseline (speedup 1.0000x reference)
"""Trainium2 Bass kernel for nn_DiscriminatorWithLS4.

The reference model only consumes the LAST timestep of the LS4 scan output
(``z[:, -1, :]``), so the diagonal linear recurrence

    h_t = a * h_{t-1} + B * u_t,   y_t = sum_n C * h_t + D * u_t

collapses in closed form to a fixed weighted reduction over time:

    y_T[b,d] = sum_t Keff[t,d] * u[b,t,d]
    Keff[t,d] = sum_n C[d,n] B[d,n] a[d,n]^(T-1-t)   (+ D[d] at t = T-1)
    u[b,t,d]  = sum_c in_chan[c,b,t] * mask[b,c] * W_in[c,d] + b_in[d]
    mask[b,c] = in_chan[c,b,T-1]

Keff is a pure parameter transform, computed host-side in f64.  Because
a = sigmoid(log_a) < 1 elementwise, |Keff[t]| decays geometrically going
back in time; only the trailing window with non-negligible mass is streamed.
The output linears collapse as well: only gelu(y_T) @ W_mu @ W_lin is
needed, so W_mu @ W_lin ([d,1]) and W_lin . b_mu + b_lin fold on the host,
as does MW[d,(b,c)] = mask[b,c] * W_in[c,d] (parameter-sized products).

Device work per core (data-parallel over batch, 8 batches/core, no
collectives):

    P^T[d,r]  = sum_t Keff[t,d] * X[t,r]        PE: accumulate 128-t chunks
    q         = P^T * MW                        DVE (MW streamed in blob)
    y^T[d,b]  = sum_c q[d,(b,c)]                DVE contiguous reduce
    yg        = gelu_tanh(y^T + S*b_in)         ACT (bias fused)
    out[b]    = sigmoid(Wcombo^T @ yg + blin')  PE + ACT

Keff/X stream in bf16 (rel output error ~5e-3 against the 2e-2 gate);
the f32 param sections (MW, gelu bias) ride in the SAME bf16 blob
bit-packed as bf16 pairs and are read through f32 bitcast APs, so ONE
HWDGE DMA covers all input (a second HWDGE DMA would serialize its
descriptor generation on the single HWDGE device).  The final
sigmoid(z + blin) is an elementwise map over the [1, B, 1] output,
applied on the host during unsharding; the device ends with the readout
matvec and a PSUM->SBUF copy feeding the 32-byte output DMA.

This toolchain's walrus codegen accepts at most ONE semaphore wait per
instruction; ``_legalize_multiwaits`` splits any multi-wait instruction
into single-wait same-engine NoOps + the instruction (semantically
identical, codegen-legal).
"""

import numpy as np

C_IN, BATCH, T_FULL = 8, 64, 4096
D_MODEL, N_STATE, HID = 128, 64, 128
N_CORES = 8
B_SH = BATCH // N_CORES          # batches per core
RB = C_IN * B_SH                 # stream rows per core: (b_local, c), b outer
CK = D_MODEL + RB                # bf16 columns per chunk
# Params after the chunks: MW [128,64] and gbias [128,1] in f32 packed as
# bf16 pairs, then wcombo [128,1] in plain bf16 (+1 pad col so the row
# pitch stays even — the f32 bitcast views need an integral f32 stride).
PW = 2 * (RB + 1) + 2

_prog_cache = {}


def _legalize_multiwaits(nc):
    """Split every instruction carrying N>1 semaphore waits into N-1
    single-wait NoOps (same engine, program order preserved) followed by
    the instruction with its final wait."""
    import concourse.mybir as mybir

    for fn in nc.m.functions:
        for blk in fn.blocks:
            idx = 0
            insts = blk.instructions
            while idx < len(insts):
                inst = insts[idx]
                si = inst.sync_info
                if si is not None and len(si.on_wait) > 1:
                    waits = list(si.on_wait)
                    if inst.opcode in ("TensorTensor", "Activation", "Matmult",
                                       "TensorReduce", "TensorScalarPtr",
                                       "TriggerDma"):
                        # For compute ops, park DMA-queue waits (earliest to
                        # resolve) on the NoOps and keep an engine-sem wait
                        # (usually latest) on the instruction, so NoOps clear
                        # early instead of blocking the queue.
                        waits.sort(
                            key=lambda w: 0 if str(
                                getattr(w, "ant_name", "")
                            ).startswith(("DMASW", "DMAHW")) else 1
                        )
                    for k, w in enumerate(waits[:-1]):
                        nop = mybir.InstNoOp(
                            name=f"{inst.name}-mw{k}",
                            sync_info=mybir.SyncInfo(on_wait=[w], on_update=[]),
                            engine=inst.engine,
                            bass_nofuse=True,
                        )
                        try:
                            nc.register_instruction(nop)
                        except Exception:
                            pass
                        insts.insert(idx, nop)
                        idx += 1
                    si.on_wait = [waits[-1]]
                idx += 1


def _unwait_out_dma(nc):
    """Remove the tail drains' waits on the OUTPUT DMA's completion
    semaphore.  The instruction-level wait only guards the gap between the
    last engine instruction and kernel end; the runtime's execution-complete
    quiesce waits for the DMA rings independently (and serializes NEFF
    executions), so the engines may end their streams while the final 32-byte
    store is in flight.  The DMA's sem update is kept for ring accounting."""
    # The output DMA is the one with data-dependency waits (the input blob
    # DMA is wait-free).
    out_sems = set()
    for fn in nc.m.functions:
        for blk in fn.blocks:
            for inst in blk.instructions:
                if inst.opcode != "DMACopy":
                    continue
                si = inst.sync_info
                if si is None or not si.on_wait:
                    continue
                for u in si.on_update or []:
                    out_sems.add(str(getattr(u, "ant_name", "")))
    if not out_sems:
        return
    for fn in nc.m.functions:
        for blk in fn.blocks:
            for inst in blk.instructions:
                si = inst.sync_info
                if si is None:
                    continue
                if inst.opcode not in ("Drain", "EventSemaphore", "NoOp"):
                    continue
                if not si.on_wait:
                    continue
                si.on_wait = [
                    w for w in si.on_wait
                    if str(getattr(w, "ant_name", "")) not in out_sems
                ]


def _strip_preamble(nc):
    """Drop the Bass-init const memsets and the initial all-engine barrier
    from the first block.  The const APs are unused by this kernel and every
    cross-engine dependency is carried by the Tile-generated semaphores, so
    the barrier is dead weight before the first DMA can issue."""
    blk = nc.m.functions[0].blocks[0]
    keep = [
        i for i in blk.instructions
        if i.opcode not in ("Memset", "Drain", "EventSemaphore")
    ]
    while len(blk.instructions):
        blk.instructions.pop()
    for i in keep:
        blk.instructions.append(i)


def _trim_tail(nc):
    """Remove the second all-engine barrier after the tail semaphore-clear.
    The first barrier already guarantees every engine is past its last
    semaphore wait before the clear, and the runtime serializes NEFF
    executions, so engines may end their streams without re-synchronizing
    after the clear.  (Validated by the bit-identical re-execution check.)"""
    blk = nc.m.functions[0].blocks[-1]
    isa_idx = None
    for i, inst in enumerate(blk.instructions):
        if inst.opcode == "ISA":
            isa_idx = i
    if isa_idx is None:
        return
    while len(blk.instructions) > isa_idx + 1:
        blk.instructions.pop()


def _hoist_lead_dma(nc):
    """Move the wait-free input DMACopies (chunks on SP, params on ACT) to
    the very front of the first block, ahead of the engines' RegisterMove
    preambles, so descriptor generation starts at t~0 instead of after
    ~300-500 ns of register setup and branching.  (The DMACopies read no
    registers, so reordering past the preamble is safe.)"""
    fn = nc.m.functions[0]
    main = fn.blocks[0]
    hoisted = []
    for blk in fn.blocks[1:]:
        for inst in list(blk.instructions):
            if inst.opcode != "DMACopy":
                continue
            if not (str(inst.engine).endswith("SP")
                    or str(inst.engine).endswith("Pool")
                    or str(inst.engine).endswith("Activation")):
                continue
            si = inst.sync_info
            if si is not None and si.on_wait:
                continue
            idx = [i for i, x in enumerate(blk.instructions)
                   if x.name == inst.name]
            blk.instructions.pop(idx[0])
            hoisted.append(inst)
        break
    for inst in reversed(hoisted):
        main.instructions.insert(0, inst)


def _scrub_tracebacks(nc):
    """Blank the caller tracebacks in per-instruction debug info so the BIR
    bytes — and therefore the NEFF compile-cache key — are identical no
    matter which process or call site builds the kernel."""
    import bass_rust

    for fn in nc.m.functions:
        for blk in fn.blocks:
            for inst in blk.instructions:
                d = inst.debug
                if d is None or not getattr(d, "ant_traceback", None):
                    continue
                inst.debug = bass_rust.OpDebugInfo(
                    op_name=d.op_name,
                    tensorizer_id=d.tensorizer_id,
                    filename=d.filename,
                    lineno=d.lineno,
                    bass_funcname=d.bass_funcname,
                    kernel_name=d.kernel_name,
                    ant_traceback="",
                    ant_layer=d.ant_layer,
                    ant_annotation=d.ant_annotation,
                )


def _build_bass(teff):
    """Per-core Bass program: teff timesteps streamed in bf16, f32 params
    bit-packed in the same blob; teff=128k streams k chunks of 128
    timesteps."""
    import concourse.bass as bass
    import concourse.mybir as mybir
    import concourse.tile as tile

    f32 = mybir.dt.float32
    bf16 = mybir.dt.bfloat16
    nc = bass.Bass(disable_frame_to_traceback=True)

    assert teff % 128 == 0
    nj = teff // 128
    P0 = nj * CK
    SW = P0 + PW
    blob = nc.dram_tensor("blob", [128, SW], bf16, kind="ExternalInput")
    out = nc.dram_tensor("out", [1, B_SH], f32, kind="ExternalOutput")

    with tile.TileContext(nc) as tc:
        with (
            tc.tile_pool(name="stream", bufs=1) as stream,
            tc.tile_pool(name="work", bufs=1) as work,
            tc.tile_pool(name="psum", bufs=1, space="PSUM") as psum,
        ):
            # One DMA for everything: a second HWDGE DMA would serialize its
            # descriptor generation (single HWDGE device) and rows under
            # 512 B pay a 2x bus penalty — merged rows stay above it.
            blob_sb = stream.tile([128, SW], bf16)
            nc.sync.dma_start(out=blob_sb, in_=blob[:, :])

            # f32 param views into the bf16 blob (bit-packed pairs); wcombo
            # stays plain bf16 so the final matvec runs at 1 cycle/row.
            mw_ap = blob_sb[:, P0:P0 + 2 * RB].bitcast(f32)
            gbias_ap = blob_sb[:, P0 + 2 * RB:P0 + 2 * RB + 2].bitcast(f32)
            wcombo_ap = blob_sb[:, P0 + 2 * RB + 2:P0 + 2 * RB + 3]

            # --- PE: P^T[d, r] = sum_t Keff[t, d] * X[t, r] ---
            pT_ps = psum.tile([D_MODEL, RB], f32)
            for j in range(nj):
                nc.tensor.matmul(
                    pT_ps[:, :],
                    lhsT=blob_sb[:, j * CK:j * CK + D_MODEL],
                    rhs=blob_sb[:, j * CK + D_MODEL:(j + 1) * CK],
                    start=(j == 0),
                    stop=(j == nj - 1),
                )

            # q = P^T * MW;  y^T[d, b] = sum_c q[d, (b, c)]
            q_sb = work.tile([D_MODEL, RB], f32)
            nc.vector.tensor_mul(out=q_sb[:, :], in0=pT_ps[:, :], in1=mw_ap)
            y_sb = work.tile([D_MODEL, B_SH], f32)
            nc.vector.tensor_reduce(
                out=y_sb[:, :],
                in_=q_sb.rearrange("p (b c) -> p b c", c=C_IN),
                axis=mybir.AxisListType.X,
                op=mybir.AluOpType.add,
            )

            # yg = gelu_tanh(y + S*b_in)  (bias fused; jax.nn.gelu default
            # is the tanh approximation).  bf16 out so the matvec runs at
            # bf16 rate.
            yg_sb = work.tile([D_MODEL, B_SH], bf16)
            nc.scalar.activation(
                out=yg_sb[:, :],
                in_=y_sb[:, :],
                func=mybir.ActivationFunctionType.Gelu_apprx_tanh,
                bias=gbias_ap,
            )

            # z[b] = Wcombo^T @ yg; the final sigmoid(z + blin) is an
            # elementwise map on the [1, B] output, applied on the host
            # during unsharding.
            o_ps = psum.tile([1, B_SH], f32)
            nc.tensor.matmul(o_ps[:, :], lhsT=wcombo_ap, rhs=yg_sb[:, :])
            o_sb = work.tile([1, B_SH], f32)
            nc.vector.tensor_scalar(
                out=o_sb[:, :], in0=o_ps[:, :],
                scalar1=1.0, scalar2=0.0,
                op0=mybir.AluOpType.mult, op1=mybir.AluOpType.add,
            )
            nc.sync.dma_start(out=out[:, :], in_=o_sb[:, :])

    _unwait_out_dma(nc)
    _legalize_multiwaits(nc)
    _strip_preamble(nc)
    _hoist_lead_dma(nc)
    _trim_tail(nc)
    _scrub_tracebacks(nc)
    return nc


def _host_keff(log_a, B_ssm, C_ssm, D_ssm):
    """Keff[t, d] over the full horizon in f64, built backwards with early
    exit once the remaining mass is negligible.  Returns (Keff, S)."""
    a = 1.0 / (1.0 + np.exp(-log_a.astype(np.float64)))        # [d, N]
    cb = C_ssm.astype(np.float64) * B_ssm.astype(np.float64)   # [d, N]
    K = np.zeros((T_FULL, D_MODEL))
    p = cb.copy()
    for t in range(T_FULL - 1, -1, -1):
        K[t] = p.sum(axis=1)
        p *= a
        if np.abs(p).sum(axis=1).max() < 1e-13:
            break
    Keff = K
    Keff[T_FULL - 1] += D_ssm.astype(np.float64)
    S = Keff.sum(axis=0)
    return Keff, S


def _pick_window(Keff, wcombo):
    """Smallest window (64, or a multiple of 128) whose truncation error
    estimate is well under the 2e-2 gate.  Truncated terms sum with random
    signs, so the y error is ~rms: err_y[d] = sqrt(sum_{t<T-teff}
    Keff[t,d]^2) * rms(u) (rms(u)~1 for unit-normal data through the
    unit-scaled W_in).  Through gelu' (<=1.13), wcombo and sigmoid' (<=0.25)
    the output error is ~0.25*||err_y * wcombo||; require < 1e-3 (20x under
    the gate, and small against the ~3e-3 bf16 quantization floor)."""
    c2 = np.cumsum(Keff ** 2, axis=0)  # [T, d]

    def ok(teff):
        if teff >= T_FULL:
            return True
        err_y = np.sqrt(c2[T_FULL - teff - 1])          # [d]
        return 0.25 * 1.13 * np.linalg.norm(err_y * np.abs(wcombo)) < 1e-3

    for nj in range(1, T_FULL // 128 + 1):
        if ok(nj * 128):
            return nj * 128
    return T_FULL


_runner_cache = {}


def _get_cached_runner(nc, key):
    """Build the sharded PJRT callable for `nc` once and reuse it across
    kernel() calls — run_bass_kernel_spmd re-traces and re-jits the wrapper
    on every invocation (~0.3 s of host time)."""
    if key in _runner_cache:
        return _runner_cache[key]

    import jax
    import numpy as _np
    from jax.experimental.shard_map import shard_map
    from jax.sharding import Mesh, PartitionSpec
    import concourse.mybir as mybir
    from concourse.bass2jax import (
        _bass_exec_p,
        install_neuronx_cc_hook,
        partition_id_tensor,
    )

    install_neuronx_cc_hook()
    assert nc.dbg_addr is None
    partition_name = (
        nc.partition_id_tensor.name if nc.partition_id_tensor else None
    )

    in_names, out_names, out_avals = [], [], []
    for alloc in nc.m.functions[0].allocations:
        if not isinstance(alloc, mybir.MemoryLocationSet):
            continue
        name = alloc.memorylocations[0].name
        if alloc.kind == "ExternalInput":
            if name != partition_name:
                in_names.append(name)
        elif alloc.kind == "ExternalOutput":
            out_names.append(name)
            out_avals.append(
                jax.core.ShapedArray(
                    tuple(alloc.tensor_shape), mybir.dt.np(alloc.dtype)
                )
            )
    n_params = len(in_names)
    all_names = list(in_names) + list(out_names)
    if partition_name is not None:
        all_names.append(partition_name)
    all_names = tuple(all_names)
    donate = tuple(range(n_params, n_params + len(out_names)))

    def _body(*args):
        operands = list(args)
        if partition_name is not None:
            operands.append(partition_id_tensor())
        outs = _bass_exec_p.bind(
            *operands,
            out_avals=tuple(out_avals),
            in_names=all_names,
            out_names=tuple(out_names),
            lowering_input_output_aliases=(),
            sim_require_finite=True,
            sim_require_nnan=True,
            nc=nc,
        )
        return tuple(outs)

    devices = jax.devices()[:N_CORES]
    mesh = Mesh(_np.asarray(devices), ("core",))
    specs = (PartitionSpec("core"),) * (n_params + len(out_names))
    sharded = jax.jit(
        shard_map(
            _body, mesh=mesh, in_specs=specs,
            out_specs=(PartitionSpec("core"),) * len(out_names),
            check_rep=False,
        ),
        donate_argnums=donate,
        keep_unused=True,
    )

    def run(in_maps):
        concat_in = [
            np.concatenate([in_maps[c][n] for c in range(N_CORES)], axis=0)
            for n in in_names
        ]
        concat_zeros = [
            np.zeros((N_CORES * a.shape[0], *a.shape[1:]), a.dtype)
            for a in out_avals
        ]
        out_arrs = sharded(*concat_in, *concat_zeros)
        return [
            {
                n: np.asarray(out_arrs[i]).reshape(
                    N_CORES, *out_avals[i].shape
                )[c]
                for i, n in enumerate(out_names)
            }
            for c in range(N_CORES)
        ]

    _runner_cache[key] = run
    return run


def kernel(**inputs):
    from concourse.bass_utils import run_bass_kernel_spmd
    import ml_dtypes

    bf16 = ml_dtypes.bfloat16

    in_chan = np.ascontiguousarray(np.asarray(inputs["in_chan"], dtype=np.float32))
    W_in = np.asarray(inputs["W_in"], dtype=np.float32)
    b_in = np.asarray(inputs["b_in"], dtype=np.float32)
    log_a = np.asarray(inputs["log_a"], dtype=np.float32)
    B_ssm = np.asarray(inputs["B_ssm"], dtype=np.float32)
    C_ssm = np.asarray(inputs["C_ssm"], dtype=np.float32)
    D_ssm = np.asarray(inputs["D_ssm"], dtype=np.float32)
    W_mu = np.asarray(inputs["W_mu"], dtype=np.float32)
    b_mu = np.asarray(inputs["b_mu"], dtype=np.float32)
    W_lin = np.asarray(inputs["W_lin"], dtype=np.float32)
    b_lin = np.asarray(inputs["b_lin"], dtype=np.float32)

    Keff, S = _host_keff(log_a, B_ssm, C_ssm, D_ssm)
    wcombo = (W_mu @ W_lin).astype(np.float32)                 # [d, 1]
    teff = _pick_window(Keff, wcombo[:, 0])
    nj = teff // 128
    P0 = nj * CK

    # Device-layout param sections (per-core MW differs; rest shared).
    kw = Keff[T_FULL - teff:].astype(bf16)                     # [teff, d]
    blin_eff = np.float32(W_lin[:, 0] @ b_mu + b_lin[0])
    gbias = (b_in * S.astype(np.float32)).astype(np.float32)

    mask = in_chan[:, :, T_FULL - 1]                           # [C, B]
    win = in_chan[:, :, T_FULL - teff:]                        # [C, B, teff]

    in_maps = []
    for core in range(N_CORES):
        sl = win[:, core * B_SH:(core + 1) * B_SH, :]          # [C, B_SH, teff]
        xt = sl.transpose(2, 1, 0).reshape(teff, RB).astype(bf16)
        # MW[d, (b, c)] = mask[b, c] * W_in[c, d]   (parameter-sized fold)
        m_bc = mask[:, core * B_SH:(core + 1) * B_SH].T         # [B_SH, C]
        mw = (m_bc[:, :, None] * W_in[:, None, :].transpose(1, 0, 2))
        # mw[b, c, d] -> [d, (b, c)]
        mw = np.ascontiguousarray(
            mw.reshape(RB, D_MODEL).T, dtype=np.float32
        )                                                       # [128, 64]

        params32 = np.zeros((D_MODEL, RB + 1), dtype=np.float32)
        params32[:, 0:RB] = mw
        params32[:, RB] = gbias

        blob = np.zeros((128, P0 + PW), dtype=bf16)
        kw_c = kw.reshape(nj, 128, D_MODEL).transpose(1, 0, 2)
        xt_c = xt.reshape(nj, 128, RB).transpose(1, 0, 2)
        for j in range(nj):
            blob[:, j * CK:j * CK + D_MODEL] = kw_c[:, j]
            blob[:, j * CK + D_MODEL:(j + 1) * CK] = xt_c[:, j]
        blob[:, P0:P0 + 2 * (RB + 1)] = params32.view(np.uint16).view(bf16)
        blob[:, P0 + 2 * (RB + 1)] = wcombo[:, 0].astype(bf16)
        in_maps.append({"blob": blob})

    key = teff
    if key not in _prog_cache:
        _prog_cache[key] = _build_bass(teff)
    nc = _prog_cache[key]

    try:
        results = _get_cached_runner(nc, key)(in_maps)
    except Exception:
        _runner_cache.pop(key, None)
        results = run_bass_kernel_spmd(
            nc, in_maps, core_ids=list(range(N_CORES))
        ).results
    outs = [results[c]["out"][0, :] for c in range(N_CORES)]
    z = np.concatenate(outs).astype(np.float32) + blin_eff
    full = (1.0 / (1.0 + np.exp(-z))).reshape(1, BATCH, 1).astype(np.float32)
    return full


# revision 45
# speedup vs baseline: 1.0808x; 1.0808x over previous
"""Trainium2 Bass kernel for nn_DiscriminatorWithLS4.

The reference model only consumes the LAST timestep of the LS4 scan output
(``z[:, -1, :]``), so the diagonal linear recurrence

    h_t = a * h_{t-1} + B * u_t,   y_t = sum_n C * h_t + D * u_t

collapses in closed form to a fixed weighted reduction over time:

    y_T[b,d] = sum_t Keff[t,d] * u[b,t,d]
    Keff[t,d] = sum_n C[d,n] B[d,n] a[d,n]^(T-1-t)   (+ D[d] at t = T-1)
    u[b,t,d]  = sum_c in_chan[c,b,t] * mask[b,c] * W_in[c,d] + b_in[d]
    mask[b,c] = in_chan[c,b,T-1]

Keff is a pure parameter transform, computed host-side in f64.  Because
a = sigmoid(log_a) < 1 elementwise, |Keff[t]| decays geometrically going
back in time; only the trailing window with non-negligible mass is streamed.
The output linears collapse as well: only gelu(y_T) @ W_mu @ W_lin is
needed, so W_mu @ W_lin ([d,1]) and W_lin . b_mu + b_lin fold on the host,
as does MW[d,(b,c)] = mask[b,c] * W_in[c,d] (parameter-sized products).

Device work per core (data-parallel over batch, 8 batches/core, no
collectives):

    P^T[d,r]  = sum_t Keff[t,d] * X[t,r]        PE: accumulate 128-t chunks
    q         = P^T * MW                        DVE (MW streamed in blob)
    y^T[d,b]  = sum_c q[d,(b,c)]                DVE contiguous reduce
    yg        = gelu_tanh(y^T + S*b_in)         ACT (bias fused)
    out[b]    = sigmoid(Wcombo^T @ yg + blin')  PE + ACT

Keff/X stream in bf16 (rel output error ~5e-3 against the 2e-2 gate);
the f32 param sections (MW, gelu bias) ride in the SAME bf16 blob
bit-packed as bf16 pairs and are read through f32 bitcast APs, so ONE
HWDGE DMA covers all input (a second HWDGE DMA would serialize its
descriptor generation on the single HWDGE device).  The final
sigmoid(z + blin) is an elementwise map over the [1, B, 1] output,
applied on the host during unsharding; the device ends with the readout
matvec and a PSUM->SBUF copy feeding the 32-byte output DMA.

This toolchain's walrus codegen accepts at most ONE semaphore wait per
instruction; ``_legalize_multiwaits`` splits any multi-wait instruction
into single-wait same-engine NoOps + the instruction (semantically
identical, codegen-legal).
"""

import numpy as np

C_IN, BATCH, T_FULL = 8, 64, 4096
D_MODEL, N_STATE, HID = 128, 64, 128
N_CORES = 8
B_SH = BATCH // N_CORES          # batches per core
RB = C_IN * B_SH                 # stream rows per core: (b_local, c), b outer
CK = D_MODEL + RB                # bf16 columns per chunk
# Params after the chunks: MW [128,64] and gbias [128,1] in f32 packed as
# bf16 pairs (the even count keeps the row pitch f32-alignable for the
# bitcast views).
PW = 2 * (RB + 1)

_prog_cache = {}


def _legalize_multiwaits(nc):
    """Split every instruction carrying N>1 semaphore waits into N-1
    single-wait NoOps (same engine, program order preserved) followed by
    the instruction with its final wait."""
    import concourse.mybir as mybir

    for fn in nc.m.functions:
        for blk in fn.blocks:
            idx = 0
            insts = blk.instructions
            while idx < len(insts):
                inst = insts[idx]
                si = inst.sync_info
                if si is not None and len(si.on_wait) > 1:
                    waits = list(si.on_wait)
                    if inst.opcode in ("TensorTensor", "Activation", "Matmult",
                                       "TensorReduce", "TensorScalarPtr",
                                       "TriggerDma"):
                        # For compute ops, park DMA-queue waits (earliest to
                        # resolve) on the NoOps and keep an engine-sem wait
                        # (usually latest) on the instruction, so NoOps clear
                        # early instead of blocking the queue.
                        waits.sort(
                            key=lambda w: 0 if str(
                                getattr(w, "ant_name", "")
                            ).startswith(("DMASW", "DMAHW")) else 1
                        )
                    for k, w in enumerate(waits[:-1]):
                        nop = mybir.InstNoOp(
                            name=f"{inst.name}-mw{k}",
                            sync_info=mybir.SyncInfo(on_wait=[w], on_update=[]),
                            engine=inst.engine,
                            bass_nofuse=True,
                        )
                        try:
                            nc.register_instruction(nop)
                        except Exception:
                            pass
                        insts.insert(idx, nop)
                        idx += 1
                    si.on_wait = [waits[-1]]
                idx += 1


def _unwait_out_dma(nc):
    """Remove the tail drains' waits on the OUTPUT DMA's completion
    semaphore.  The instruction-level wait only guards the gap between the
    last engine instruction and kernel end; the runtime's execution-complete
    quiesce waits for the DMA rings independently (and serializes NEFF
    executions), so the engines may end their streams while the final 32-byte
    store is in flight.  The DMA's sem update is kept for ring accounting."""
    # The output DMA is the one with data-dependency waits (the input blob
    # DMA is wait-free).
    out_sems = set()
    for fn in nc.m.functions:
        for blk in fn.blocks:
            for inst in blk.instructions:
                if inst.opcode != "DMACopy":
                    continue
                si = inst.sync_info
                if si is None or not si.on_wait:
                    continue
                for u in si.on_update or []:
                    out_sems.add(str(getattr(u, "ant_name", "")))
    if not out_sems:
        return
    for fn in nc.m.functions:
        for blk in fn.blocks:
            for inst in blk.instructions:
                si = inst.sync_info
                if si is None:
                    continue
                if inst.opcode not in ("Drain", "EventSemaphore", "NoOp"):
                    continue
                if not si.on_wait:
                    continue
                si.on_wait = [
                    w for w in si.on_wait
                    if str(getattr(w, "ant_name", "")) not in out_sems
                ]


def _strip_preamble(nc):
    """Drop the Bass-init const memsets and the initial all-engine barrier
    from the first block.  The const APs are unused by this kernel and every
    cross-engine dependency is carried by the Tile-generated semaphores, so
    the barrier is dead weight before the first DMA can issue."""
    blk = nc.m.functions[0].blocks[0]
    keep = [
        i for i in blk.instructions
        if i.opcode not in ("Memset", "Drain", "EventSemaphore")
    ]
    while len(blk.instructions):
        blk.instructions.pop()
    for i in keep:
        blk.instructions.append(i)


def _trim_tail(nc):
    """Remove the second all-engine barrier after the tail semaphore-clear.
    The first barrier already guarantees every engine is past its last
    semaphore wait before the clear, and the runtime serializes NEFF
    executions, so engines may end their streams without re-synchronizing
    after the clear.  (Validated by the bit-identical re-execution check.)"""
    blk = nc.m.functions[0].blocks[-1]
    isa_idx = None
    for i, inst in enumerate(blk.instructions):
        if inst.opcode == "ISA":
            isa_idx = i
    if isa_idx is None:
        return
    while len(blk.instructions) > isa_idx + 1:
        blk.instructions.pop()


def _hoist_lead_dma(nc):
    """Move the wait-free input DMACopies (chunks on SP, params on ACT) to
    the very front of the first block, ahead of the engines' RegisterMove
    preambles, so descriptor generation starts at t~0 instead of after
    ~300-500 ns of register setup and branching.  (The DMACopies read no
    registers, so reordering past the preamble is safe.)"""
    fn = nc.m.functions[0]
    main = fn.blocks[0]
    hoisted = []
    for blk in fn.blocks[1:]:
        for inst in list(blk.instructions):
            if inst.opcode != "DMACopy":
                continue
            if not (str(inst.engine).endswith("SP")
                    or str(inst.engine).endswith("Pool")
                    or str(inst.engine).endswith("Activation")):
                continue
            si = inst.sync_info
            if si is not None and si.on_wait:
                continue
            idx = [i for i, x in enumerate(blk.instructions)
                   if x.name == inst.name]
            blk.instructions.pop(idx[0])
            hoisted.append(inst)
        break
    for inst in reversed(hoisted):
        main.instructions.insert(0, inst)


def _scrub_tracebacks(nc):
    """Blank the caller tracebacks in per-instruction debug info so the BIR
    bytes — and therefore the NEFF compile-cache key — are identical no
    matter which process or call site builds the kernel."""
    import bass_rust

    for fn in nc.m.functions:
        for blk in fn.blocks:
            for inst in blk.instructions:
                d = inst.debug
                if d is None or not getattr(d, "ant_traceback", None):
                    continue
                inst.debug = bass_rust.OpDebugInfo(
                    op_name=d.op_name,
                    tensorizer_id=d.tensorizer_id,
                    filename=d.filename,
                    lineno=d.lineno,
                    bass_funcname=d.bass_funcname,
                    kernel_name=d.kernel_name,
                    ant_traceback="",
                    ant_layer=d.ant_layer,
                    ant_annotation=d.ant_annotation,
                )


def _build_bass(teff):
    """Per-core Bass program: teff timesteps streamed in bf16, f32 params
    bit-packed in the same blob; teff=128k streams k chunks of 128
    timesteps."""
    import concourse.bass as bass
    import concourse.mybir as mybir
    import concourse.tile as tile

    f32 = mybir.dt.float32
    bf16 = mybir.dt.bfloat16
    nc = bass.Bass(disable_frame_to_traceback=True)

    assert teff % 128 == 0
    nj = teff // 128
    P0 = nj * CK
    SW = P0 + PW
    blob = nc.dram_tensor("blob", [128, SW], bf16, kind="ExternalInput")
    out = nc.dram_tensor("out", [D_MODEL, B_SH], bf16, kind="ExternalOutput")

    with tile.TileContext(nc) as tc:
        with (
            tc.tile_pool(name="stream", bufs=1) as stream,
            tc.tile_pool(name="work", bufs=1) as work,
            tc.tile_pool(name="psum", bufs=1, space="PSUM") as psum,
        ):
            # One DMA for everything: a second HWDGE DMA would serialize its
            # descriptor generation (single HWDGE device) and rows under
            # 512 B pay a 2x bus penalty — merged rows stay above it.
            blob_sb = stream.tile([128, SW], bf16)
            nc.sync.dma_start(out=blob_sb, in_=blob[:, :])

            # f32 param views into the bf16 blob (bit-packed pairs)
            mw_ap = blob_sb[:, P0:P0 + 2 * RB].bitcast(f32)
            gbias_ap = blob_sb[:, P0 + 2 * RB:P0 + 2 * RB + 2].bitcast(f32)

            # --- PE: P^T[d, r] = sum_t Keff[t, d] * X[t, r] ---
            pT_ps = psum.tile([D_MODEL, RB], f32)
            for j in range(nj):
                nc.tensor.matmul(
                    pT_ps[:, :],
                    lhsT=blob_sb[:, j * CK:j * CK + D_MODEL],
                    rhs=blob_sb[:, j * CK + D_MODEL:(j + 1) * CK],
                    start=(j == 0),
                    stop=(j == nj - 1),
                )

            # q = P^T * MW;  y^T[d, b] = sum_c q[d, (b, c)]
            q_sb = work.tile([D_MODEL, RB], f32)
            nc.vector.tensor_mul(out=q_sb[:, :], in0=pT_ps[:, :], in1=mw_ap)
            y_sb = work.tile([D_MODEL, B_SH], f32)
            nc.vector.tensor_reduce(
                out=y_sb[:, :],
                in_=q_sb.rearrange("p (b c) -> p b c", c=C_IN),
                axis=mybir.AxisListType.X,
                op=mybir.AluOpType.add,
            )

            # yg = gelu_tanh(y + S*b_in)  (bias fused; jax.nn.gelu default
            # is the tanh approximation).  yg [d, b] streams straight out;
            # the 128-weight readout head (z = Wcombo . yg + blin, then
            # sigmoid) is applied on the host during unsharding — it would
            # otherwise cost a PE matvec + PSUM->SBUF copy + two semaphore
            # hops before the output DMA could issue.
            yg_sb = work.tile([D_MODEL, B_SH], bf16)
            nc.scalar.activation(
                out=yg_sb[:, :],
                in_=y_sb[:, :],
                func=mybir.ActivationFunctionType.Gelu_apprx_tanh,
                bias=gbias_ap,
            )
            nc.sync.dma_start(out=out[:, :], in_=yg_sb[:, :])

    _unwait_out_dma(nc)
    _legalize_multiwaits(nc)
    _strip_preamble(nc)
    _hoist_lead_dma(nc)
    _trim_tail(nc)
    _scrub_tracebacks(nc)
    return nc


def _host_keff(log_a, B_ssm, C_ssm, D_ssm):
    """Keff[t, d] over the full horizon in f64, built backwards with early
    exit once the remaining mass is negligible.  Returns (Keff, S)."""
    a = 1.0 / (1.0 + np.exp(-log_a.astype(np.float64)))        # [d, N]
    cb = C_ssm.astype(np.float64) * B_ssm.astype(np.float64)   # [d, N]
    K = np.zeros((T_FULL, D_MODEL))
    p = cb.copy()
    for t in range(T_FULL - 1, -1, -1):
        K[t] = p.sum(axis=1)
        p *= a
        if np.abs(p).sum(axis=1).max() < 1e-13:
            break
    Keff = K
    Keff[T_FULL - 1] += D_ssm.astype(np.float64)
    S = Keff.sum(axis=0)
    return Keff, S


def _pick_window(Keff, wcombo):
    """Smallest window (64, or a multiple of 128) whose truncation error
    estimate is well under the 2e-2 gate.  Truncated terms sum with random
    signs, so the y error is ~rms: err_y[d] = sqrt(sum_{t<T-teff}
    Keff[t,d]^2) * rms(u) (rms(u)~1 for unit-normal data through the
    unit-scaled W_in).  Through gelu' (<=1.13), wcombo and sigmoid' (<=0.25)
    the output error is ~0.25*||err_y * wcombo||; require < 1e-3 (20x under
    the gate, and small against the ~3e-3 bf16 quantization floor)."""
    c2 = np.cumsum(Keff ** 2, axis=0)  # [T, d]

    def ok(teff):
        if teff >= T_FULL:
            return True
        err_y = np.sqrt(c2[T_FULL - teff - 1])          # [d]
        return 0.25 * 1.13 * np.linalg.norm(err_y * np.abs(wcombo)) < 1e-3

    for nj in range(1, T_FULL // 128 + 1):
        if ok(nj * 128):
            return nj * 128
    return T_FULL


_runner_cache = {}


def _get_cached_runner(nc, key):
    """Build the sharded PJRT callable for `nc` once and reuse it across
    kernel() calls — run_bass_kernel_spmd re-traces and re-jits the wrapper
    on every invocation (~0.3 s of host time)."""
    if key in _runner_cache:
        return _runner_cache[key]

    import jax
    import numpy as _np
    from jax.experimental.shard_map import shard_map
    from jax.sharding import Mesh, PartitionSpec
    import concourse.mybir as mybir
    from concourse.bass2jax import (
        _bass_exec_p,
        install_neuronx_cc_hook,
        partition_id_tensor,
    )

    install_neuronx_cc_hook()
    assert nc.dbg_addr is None
    partition_name = (
        nc.partition_id_tensor.name if nc.partition_id_tensor else None
    )

    in_names, out_names, out_avals = [], [], []
    for alloc in nc.m.functions[0].allocations:
        if not isinstance(alloc, mybir.MemoryLocationSet):
            continue
        name = alloc.memorylocations[0].name
        if alloc.kind == "ExternalInput":
            if name != partition_name:
                in_names.append(name)
        elif alloc.kind == "ExternalOutput":
            out_names.append(name)
            out_avals.append(
                jax.core.ShapedArray(
                    tuple(alloc.tensor_shape), mybir.dt.np(alloc.dtype)
                )
            )
    n_params = len(in_names)
    all_names = list(in_names) + list(out_names)
    if partition_name is not None:
        all_names.append(partition_name)
    all_names = tuple(all_names)
    donate = tuple(range(n_params, n_params + len(out_names)))

    def _body(*args):
        operands = list(args)
        if partition_name is not None:
            operands.append(partition_id_tensor())
        outs = _bass_exec_p.bind(
            *operands,
            out_avals=tuple(out_avals),
            in_names=all_names,
            out_names=tuple(out_names),
            lowering_input_output_aliases=(),
            sim_require_finite=True,
            sim_require_nnan=True,
            nc=nc,
        )
        return tuple(outs)

    devices = jax.devices()[:N_CORES]
    mesh = Mesh(_np.asarray(devices), ("core",))
    specs = (PartitionSpec("core"),) * (n_params + len(out_names))
    sharded = jax.jit(
        shard_map(
            _body, mesh=mesh, in_specs=specs,
            out_specs=(PartitionSpec("core"),) * len(out_names),
            check_rep=False,
        ),
        donate_argnums=donate,
        keep_unused=True,
    )

    def run(in_maps):
        concat_in = [
            np.concatenate([in_maps[c][n] for c in range(N_CORES)], axis=0)
            for n in in_names
        ]
        concat_zeros = [
            np.zeros((N_CORES * a.shape[0], *a.shape[1:]), a.dtype)
            for a in out_avals
        ]
        out_arrs = sharded(*concat_in, *concat_zeros)
        return [
            {
                n: np.asarray(out_arrs[i]).reshape(
                    N_CORES, *out_avals[i].shape
                )[c]
                for i, n in enumerate(out_names)
            }
            for c in range(N_CORES)
        ]

    _runner_cache[key] = run
    return run


def kernel(**inputs):
    from concourse.bass_utils import run_bass_kernel_spmd
    import ml_dtypes

    bf16 = ml_dtypes.bfloat16

    in_chan = np.ascontiguousarray(np.asarray(inputs["in_chan"], dtype=np.float32))
    W_in = np.asarray(inputs["W_in"], dtype=np.float32)
    b_in = np.asarray(inputs["b_in"], dtype=np.float32)
    log_a = np.asarray(inputs["log_a"], dtype=np.float32)
    B_ssm = np.asarray(inputs["B_ssm"], dtype=np.float32)
    C_ssm = np.asarray(inputs["C_ssm"], dtype=np.float32)
    D_ssm = np.asarray(inputs["D_ssm"], dtype=np.float32)
    W_mu = np.asarray(inputs["W_mu"], dtype=np.float32)
    b_mu = np.asarray(inputs["b_mu"], dtype=np.float32)
    W_lin = np.asarray(inputs["W_lin"], dtype=np.float32)
    b_lin = np.asarray(inputs["b_lin"], dtype=np.float32)

    Keff, S = _host_keff(log_a, B_ssm, C_ssm, D_ssm)
    wcombo = (W_mu @ W_lin).astype(np.float32)                 # [d, 1]
    teff = _pick_window(Keff, wcombo[:, 0])
    nj = teff // 128
    P0 = nj * CK

    # Device-layout param sections (per-core MW differs; rest shared).
    kw = Keff[T_FULL - teff:].astype(bf16)                     # [teff, d]
    blin_eff = np.float32(W_lin[:, 0] @ b_mu + b_lin[0])
    gbias = (b_in * S.astype(np.float32)).astype(np.float32)

    mask = in_chan[:, :, T_FULL - 1]                           # [C, B]
    win = in_chan[:, :, T_FULL - teff:]                        # [C, B, teff]

    in_maps = []
    for core in range(N_CORES):
        sl = win[:, core * B_SH:(core + 1) * B_SH, :]          # [C, B_SH, teff]
        xt = sl.transpose(2, 1, 0).reshape(teff, RB).astype(bf16)
        # MW[d, (b, c)] = mask[b, c] * W_in[c, d]   (parameter-sized fold)
        m_bc = mask[:, core * B_SH:(core + 1) * B_SH].T         # [B_SH, C]
        mw = (m_bc[:, :, None] * W_in[:, None, :].transpose(1, 0, 2))
        # mw[b, c, d] -> [d, (b, c)]
        mw = np.ascontiguousarray(
            mw.reshape(RB, D_MODEL).T, dtype=np.float32
        )                                                       # [128, 64]

        params32 = np.zeros((D_MODEL, RB + 1), dtype=np.float32)
        params32[:, 0:RB] = mw
        params32[:, RB] = gbias

        blob = np.zeros((128, P0 + PW), dtype=bf16)
        kw_c = kw.reshape(nj, 128, D_MODEL).transpose(1, 0, 2)
        xt_c = xt.reshape(nj, 128, RB).transpose(1, 0, 2)
        for j in range(nj):
            blob[:, j * CK:j * CK + D_MODEL] = kw_c[:, j]
            blob[:, j * CK + D_MODEL:(j + 1) * CK] = xt_c[:, j]
        blob[:, P0:P0 + 2 * (RB + 1)] = params32.view(np.uint16).view(bf16)
        in_maps.append({"blob": blob})

    key = teff
    if key not in _prog_cache:
        _prog_cache[key] = _build_bass(teff)
    nc = _prog_cache[key]

    try:
        results = _get_cached_runner(nc, key)(in_maps)
    except Exception:
        _runner_cache.pop(key, None)
        results = run_bass_kernel_spmd(
            nc, in_maps, core_ids=list(range(N_CORES))
        ).results
    # Readout head on the gathered features: z = Wcombo . yg + blin, then
    # the final sigmoid.  yg is [d, B_SH] per core.
    yg = np.concatenate(
        [results[c]["out"].astype(np.float32) for c in range(N_CORES)], axis=1
    )                                                          # [d, B]
    z = wcombo[:, 0] @ yg + blin_eff
    full = (1.0 / (1.0 + np.exp(-z))).reshape(1, BATCH, 1).astype(np.float32)
    return full


# revision 46
# speedup vs baseline: 1.0893x; 1.0079x over previous
"""Trainium2 Bass kernel for nn_DiscriminatorWithLS4.

The reference model only consumes the LAST timestep of the LS4 scan output
(``z[:, -1, :]``), so the diagonal linear recurrence

    h_t = a * h_{t-1} + B * u_t,   y_t = sum_n C * h_t + D * u_t

collapses in closed form to a fixed weighted reduction over time:

    y_T[b,d] = sum_t Keff[t,d] * u[b,t,d]
    Keff[t,d] = sum_n C[d,n] B[d,n] a[d,n]^(T-1-t)   (+ D[d] at t = T-1)
    u[b,t,d]  = sum_c in_chan[c,b,t] * mask[b,c] * W_in[c,d] + b_in[d]
    mask[b,c] = in_chan[c,b,T-1]

Keff is a pure parameter transform, computed host-side in f64.  Because
a = sigmoid(log_a) < 1 elementwise, |Keff[t]| decays geometrically going
back in time; only the trailing window with non-negligible mass is streamed.
The output linears collapse as well: only gelu(y_T) @ W_mu @ W_lin is
needed, so W_mu @ W_lin ([d,1]) and W_lin . b_mu + b_lin fold on the host,
as does MW[d,(b,c)] = mask[b,c] * W_in[c,d] (parameter-sized products).

Device work per core (data-parallel over batch, 8 batches/core, no
collectives):

    P^T[d,r]  = sum_t Keff[t,d] * X[t,r]        PE: accumulate 128-t chunks
    q         = P^T * MW                        DVE (MW streamed in blob)
    y^T[d,b]  = sum_c q[d,(b,c)]                DVE contiguous reduce
    yg        = gelu_tanh(y^T + S*b_in)         ACT (bias fused)
    out[b]    = sigmoid(Wcombo^T @ yg + blin')  PE + ACT

Keff/X stream in bf16 (rel output error ~5e-3 against the 2e-2 gate);
the f32 param sections (MW, gelu bias) ride in the SAME bf16 blob
bit-packed as bf16 pairs and are read through f32 bitcast APs, so ONE
HWDGE DMA covers all input (a second HWDGE DMA would serialize its
descriptor generation on the single HWDGE device).  The final
sigmoid(z + blin) is an elementwise map over the [1, B, 1] output,
applied on the host during unsharding; the device ends with the readout
matvec and a PSUM->SBUF copy feeding the 32-byte output DMA.

This toolchain's walrus codegen accepts at most ONE semaphore wait per
instruction; ``_legalize_multiwaits`` splits any multi-wait instruction
into single-wait same-engine NoOps + the instruction (semantically
identical, codegen-legal).
"""

import numpy as np

C_IN, BATCH, T_FULL = 8, 64, 4096
D_MODEL, N_STATE, HID = 128, 64, 128
N_CORES = 8
B_SH = BATCH // N_CORES          # batches per core
RB = C_IN * B_SH                 # stream rows per core: (b_local, c), b outer
CK = D_MODEL + RB                # bf16 columns per chunk
# Params after the chunks: MW [128,64] in plain bf16, then gbias [128,1]
# in f32 packed as bf16 pairs (even total keeps the row pitch
# f32-alignable for the bitcast view).
PW = RB + 2

_prog_cache = {}


def _legalize_multiwaits(nc):
    """Split every instruction carrying N>1 semaphore waits into N-1
    single-wait NoOps (same engine, program order preserved) followed by
    the instruction with its final wait."""
    import concourse.mybir as mybir

    for fn in nc.m.functions:
        for blk in fn.blocks:
            idx = 0
            insts = blk.instructions
            while idx < len(insts):
                inst = insts[idx]
                si = inst.sync_info
                if si is not None and len(si.on_wait) > 1:
                    waits = list(si.on_wait)
                    if inst.opcode in ("TensorTensor", "Activation", "Matmult",
                                       "TensorReduce", "TensorScalarPtr",
                                       "TriggerDma"):
                        # For compute ops, park DMA-queue waits (earliest to
                        # resolve) on the NoOps and keep an engine-sem wait
                        # (usually latest) on the instruction, so NoOps clear
                        # early instead of blocking the queue.
                        waits.sort(
                            key=lambda w: 0 if str(
                                getattr(w, "ant_name", "")
                            ).startswith(("DMASW", "DMAHW")) else 1
                        )
                    for k, w in enumerate(waits[:-1]):
                        nop = mybir.InstNoOp(
                            name=f"{inst.name}-mw{k}",
                            sync_info=mybir.SyncInfo(on_wait=[w], on_update=[]),
                            engine=inst.engine,
                            bass_nofuse=True,
                        )
                        try:
                            nc.register_instruction(nop)
                        except Exception:
                            pass
                        insts.insert(idx, nop)
                        idx += 1
                    si.on_wait = [waits[-1]]
                idx += 1


def _unwait_out_dma(nc):
    """Remove the tail drains' waits on the OUTPUT DMA's completion
    semaphore.  The instruction-level wait only guards the gap between the
    last engine instruction and kernel end; the runtime's execution-complete
    quiesce waits for the DMA rings independently (and serializes NEFF
    executions), so the engines may end their streams while the final 32-byte
    store is in flight.  The DMA's sem update is kept for ring accounting."""
    # The output DMA is the one with data-dependency waits (the input blob
    # DMA is wait-free).
    out_sems = set()
    for fn in nc.m.functions:
        for blk in fn.blocks:
            for inst in blk.instructions:
                if inst.opcode != "DMACopy":
                    continue
                si = inst.sync_info
                if si is None or not si.on_wait:
                    continue
                for u in si.on_update or []:
                    out_sems.add(str(getattr(u, "ant_name", "")))
    if not out_sems:
        return
    for fn in nc.m.functions:
        for blk in fn.blocks:
            for inst in blk.instructions:
                si = inst.sync_info
                if si is None:
                    continue
                if inst.opcode not in ("Drain", "EventSemaphore", "NoOp"):
                    continue
                if not si.on_wait:
                    continue
                si.on_wait = [
                    w for w in si.on_wait
                    if str(getattr(w, "ant_name", "")) not in out_sems
                ]


def _strip_preamble(nc):
    """Drop the Bass-init const memsets and the initial all-engine barrier
    from the first block.  The const APs are unused by this kernel and every
    cross-engine dependency is carried by the Tile-generated semaphores, so
    the barrier is dead weight before the first DMA can issue."""
    blk = nc.m.functions[0].blocks[0]
    keep = [
        i for i in blk.instructions
        if i.opcode not in ("Memset", "Drain", "EventSemaphore")
    ]
    while len(blk.instructions):
        blk.instructions.pop()
    for i in keep:
        blk.instructions.append(i)


def _trim_tail(nc):
    """Remove the second all-engine barrier after the tail semaphore-clear.
    The first barrier already guarantees every engine is past its last
    semaphore wait before the clear, and the runtime serializes NEFF
    executions, so engines may end their streams without re-synchronizing
    after the clear.  (Validated by the bit-identical re-execution check.)"""
    blk = nc.m.functions[0].blocks[-1]
    isa_idx = None
    for i, inst in enumerate(blk.instructions):
        if inst.opcode == "ISA":
            isa_idx = i
    if isa_idx is None:
        return
    while len(blk.instructions) > isa_idx + 1:
        blk.instructions.pop()


def _hoist_lead_dma(nc):
    """Move the wait-free input DMACopies (chunks on SP, params on ACT) to
    the very front of the first block, ahead of the engines' RegisterMove
    preambles, so descriptor generation starts at t~0 instead of after
    ~300-500 ns of register setup and branching.  (The DMACopies read no
    registers, so reordering past the preamble is safe.)"""
    fn = nc.m.functions[0]
    main = fn.blocks[0]
    hoisted = []
    for blk in fn.blocks[1:]:
        for inst in list(blk.instructions):
            if inst.opcode != "DMACopy":
                continue
            if not (str(inst.engine).endswith("SP")
                    or str(inst.engine).endswith("Pool")
                    or str(inst.engine).endswith("Activation")):
                continue
            si = inst.sync_info
            if si is not None and si.on_wait:
                continue
            idx = [i for i, x in enumerate(blk.instructions)
                   if x.name == inst.name]
            blk.instructions.pop(idx[0])
            hoisted.append(inst)
        break
    for inst in reversed(hoisted):
        main.instructions.insert(0, inst)


def _scrub_tracebacks(nc):
    """Blank the caller tracebacks in per-instruction debug info so the BIR
    bytes — and therefore the NEFF compile-cache key — are identical no
    matter which process or call site builds the kernel."""
    import bass_rust

    for fn in nc.m.functions:
        for blk in fn.blocks:
            for inst in blk.instructions:
                d = inst.debug
                if d is None or not getattr(d, "ant_traceback", None):
                    continue
                inst.debug = bass_rust.OpDebugInfo(
                    op_name=d.op_name,
                    tensorizer_id=d.tensorizer_id,
                    filename=d.filename,
                    lineno=d.lineno,
                    bass_funcname=d.bass_funcname,
                    kernel_name=d.kernel_name,
                    ant_traceback="",
                    ant_layer=d.ant_layer,
                    ant_annotation=d.ant_annotation,
                )


def _build_bass(teff):
    """Per-core Bass program: teff timesteps streamed in bf16, f32 params
    bit-packed in the same blob; teff=128k streams k chunks of 128
    timesteps."""
    import concourse.bass as bass
    import concourse.mybir as mybir
    import concourse.tile as tile

    f32 = mybir.dt.float32
    bf16 = mybir.dt.bfloat16
    nc = bass.Bass(disable_frame_to_traceback=True)

    assert teff % 128 == 0
    nj = teff // 128
    P0 = nj * CK
    SW = P0 + PW
    blob = nc.dram_tensor("blob", [128, SW], bf16, kind="ExternalInput")
    out = nc.dram_tensor("out", [D_MODEL, B_SH], bf16, kind="ExternalOutput")

    with tile.TileContext(nc) as tc:
        with (
            tc.tile_pool(name="stream", bufs=1) as stream,
            tc.tile_pool(name="work", bufs=1) as work,
            tc.tile_pool(name="psum", bufs=1, space="PSUM") as psum,
        ):
            # One DMA for everything: a second HWDGE DMA would serialize its
            # descriptor generation (single HWDGE device) and rows under
            # 512 B pay a 2x bus penalty — merged rows stay above it.
            blob_sb = stream.tile([128, SW], bf16)
            nc.sync.dma_start(out=blob_sb, in_=blob[:, :])

            # MW rides as plain bf16; gbias is f32 bit-packed (bitcast view)
            mw_ap = blob_sb[:, P0:P0 + RB]
            gbias_ap = blob_sb[:, P0 + RB:P0 + RB + 2].bitcast(f32)

            # --- PE: P^T[d, r] = sum_t Keff[t, d] * X[t, r] ---
            pT_ps = psum.tile([D_MODEL, RB], f32)
            for j in range(nj):
                nc.tensor.matmul(
                    pT_ps[:, :],
                    lhsT=blob_sb[:, j * CK:j * CK + D_MODEL],
                    rhs=blob_sb[:, j * CK + D_MODEL:(j + 1) * CK],
                    start=(j == 0),
                    stop=(j == nj - 1),
                )

            # q = P^T * MW;  y^T[d, b] = sum_c q[d, (b, c)]
            q_sb = work.tile([D_MODEL, RB], f32)
            nc.vector.tensor_mul(out=q_sb[:, :], in0=pT_ps[:, :], in1=mw_ap)
            y_sb = work.tile([D_MODEL, B_SH], f32)
            nc.vector.tensor_reduce(
                out=y_sb[:, :],
                in_=q_sb.rearrange("p (b c) -> p b c", c=C_IN),
                axis=mybir.AxisListType.X,
                op=mybir.AluOpType.add,
            )

            # yg = gelu_tanh(y + S*b_in)  (bias fused; jax.nn.gelu default
            # is the tanh approximation).  yg [d, b] streams straight out;
            # the 128-weight readout head (z = Wcombo . yg + blin, then
            # sigmoid) is applied on the host during unsharding — it would
            # otherwise cost a PE matvec + PSUM->SBUF copy + two semaphore
            # hops before the output DMA could issue.
            yg_sb = work.tile([D_MODEL, B_SH], bf16)
            nc.scalar.activation(
                out=yg_sb[:, :],
                in_=y_sb[:, :],
                func=mybir.ActivationFunctionType.Gelu_apprx_tanh,
                bias=gbias_ap,
            )
            nc.sync.dma_start(out=out[:, :], in_=yg_sb[:, :])

    _unwait_out_dma(nc)
    _legalize_multiwaits(nc)
    _strip_preamble(nc)
    _hoist_lead_dma(nc)
    _trim_tail(nc)
    _scrub_tracebacks(nc)
    return nc


def _host_keff(log_a, B_ssm, C_ssm, D_ssm):
    """Keff[t, d] over the full horizon in f64, built backwards with early
    exit once the remaining mass is negligible.  Returns (Keff, S)."""
    a = 1.0 / (1.0 + np.exp(-log_a.astype(np.float64)))        # [d, N]
    cb = C_ssm.astype(np.float64) * B_ssm.astype(np.float64)   # [d, N]
    K = np.zeros((T_FULL, D_MODEL))
    p = cb.copy()
    for t in range(T_FULL - 1, -1, -1):
        K[t] = p.sum(axis=1)
        p *= a
        if np.abs(p).sum(axis=1).max() < 1e-13:
            break
    Keff = K
    Keff[T_FULL - 1] += D_ssm.astype(np.float64)
    S = Keff.sum(axis=0)
    return Keff, S


def _pick_window(Keff, wcombo):
    """Smallest window (64, or a multiple of 128) whose truncation error
    estimate is well under the 2e-2 gate.  Truncated terms sum with random
    signs, so the y error is ~rms: err_y[d] = sqrt(sum_{t<T-teff}
    Keff[t,d]^2) * rms(u) (rms(u)~1 for unit-normal data through the
    unit-scaled W_in).  Through gelu' (<=1.13), wcombo and sigmoid' (<=0.25)
    the output error is ~0.25*||err_y * wcombo||; require < 1e-3 (20x under
    the gate, and small against the ~3e-3 bf16 quantization floor)."""
    c2 = np.cumsum(Keff ** 2, axis=0)  # [T, d]

    def ok(teff):
        if teff >= T_FULL:
            return True
        err_y = np.sqrt(c2[T_FULL - teff - 1])          # [d]
        return 0.25 * 1.13 * np.linalg.norm(err_y * np.abs(wcombo)) < 1e-3

    for nj in range(1, T_FULL // 128 + 1):
        if ok(nj * 128):
            return nj * 128
    return T_FULL


_runner_cache = {}


def _get_cached_runner(nc, key):
    """Build the sharded PJRT callable for `nc` once and reuse it across
    kernel() calls — run_bass_kernel_spmd re-traces and re-jits the wrapper
    on every invocation (~0.3 s of host time)."""
    if key in _runner_cache:
        return _runner_cache[key]

    import jax
    import numpy as _np
    from jax.experimental.shard_map import shard_map
    from jax.sharding import Mesh, PartitionSpec
    import concourse.mybir as mybir
    from concourse.bass2jax import (
        _bass_exec_p,
        install_neuronx_cc_hook,
        partition_id_tensor,
    )

    install_neuronx_cc_hook()
    assert nc.dbg_addr is None
    partition_name = (
        nc.partition_id_tensor.name if nc.partition_id_tensor else None
    )

    in_names, out_names, out_avals = [], [], []
    for alloc in nc.m.functions[0].allocations:
        if not isinstance(alloc, mybir.MemoryLocationSet):
            continue
        name = alloc.memorylocations[0].name
        if alloc.kind == "ExternalInput":
            if name != partition_name:
                in_names.append(name)
        elif alloc.kind == "ExternalOutput":
            out_names.append(name)
            out_avals.append(
                jax.core.ShapedArray(
                    tuple(alloc.tensor_shape), mybir.dt.np(alloc.dtype)
                )
            )
    n_params = len(in_names)
    all_names = list(in_names) + list(out_names)
    if partition_name is not None:
        all_names.append(partition_name)
    all_names = tuple(all_names)
    donate = tuple(range(n_params, n_params + len(out_names)))

    def _body(*args):
        operands = list(args)
        if partition_name is not None:
            operands.append(partition_id_tensor())
        outs = _bass_exec_p.bind(
            *operands,
            out_avals=tuple(out_avals),
            in_names=all_names,
            out_names=tuple(out_names),
            lowering_input_output_aliases=(),
            sim_require_finite=True,
            sim_require_nnan=True,
            nc=nc,
        )
        return tuple(outs)

    devices = jax.devices()[:N_CORES]
    mesh = Mesh(_np.asarray(devices), ("core",))
    specs = (PartitionSpec("core"),) * (n_params + len(out_names))
    sharded = jax.jit(
        shard_map(
            _body, mesh=mesh, in_specs=specs,
            out_specs=(PartitionSpec("core"),) * len(out_names),
            check_rep=False,
        ),
        donate_argnums=donate,
        keep_unused=True,
    )

    def run(in_maps):
        concat_in = [
            np.concatenate([in_maps[c][n] for c in range(N_CORES)], axis=0)
            for n in in_names
        ]
        concat_zeros = [
            np.zeros((N_CORES * a.shape[0], *a.shape[1:]), a.dtype)
            for a in out_avals
        ]
        out_arrs = sharded(*concat_in, *concat_zeros)
        return [
            {
                n: np.asarray(out_arrs[i]).reshape(
                    N_CORES, *out_avals[i].shape
                )[c]
                for i, n in enumerate(out_names)
            }
            for c in range(N_CORES)
        ]

    _runner_cache[key] = run
    return run


def kernel(**inputs):
    from concourse.bass_utils import run_bass_kernel_spmd
    import ml_dtypes

    bf16 = ml_dtypes.bfloat16

    in_chan = np.ascontiguousarray(np.asarray(inputs["in_chan"], dtype=np.float32))
    W_in = np.asarray(inputs["W_in"], dtype=np.float32)
    b_in = np.asarray(inputs["b_in"], dtype=np.float32)
    log_a = np.asarray(inputs["log_a"], dtype=np.float32)
    B_ssm = np.asarray(inputs["B_ssm"], dtype=np.float32)
    C_ssm = np.asarray(inputs["C_ssm"], dtype=np.float32)
    D_ssm = np.asarray(inputs["D_ssm"], dtype=np.float32)
    W_mu = np.asarray(inputs["W_mu"], dtype=np.float32)
    b_mu = np.asarray(inputs["b_mu"], dtype=np.float32)
    W_lin = np.asarray(inputs["W_lin"], dtype=np.float32)
    b_lin = np.asarray(inputs["b_lin"], dtype=np.float32)

    Keff, S = _host_keff(log_a, B_ssm, C_ssm, D_ssm)
    wcombo = (W_mu @ W_lin).astype(np.float32)                 # [d, 1]
    teff = _pick_window(Keff, wcombo[:, 0])
    nj = teff // 128
    P0 = nj * CK

    # Device-layout param sections (per-core MW differs; rest shared).
    kw = Keff[T_FULL - teff:].astype(bf16)                     # [teff, d]
    blin_eff = np.float32(W_lin[:, 0] @ b_mu + b_lin[0])
    gbias = (b_in * S.astype(np.float32)).astype(np.float32)

    mask = in_chan[:, :, T_FULL - 1]                           # [C, B]
    win = in_chan[:, :, T_FULL - teff:]                        # [C, B, teff]

    in_maps = []
    for core in range(N_CORES):
        sl = win[:, core * B_SH:(core + 1) * B_SH, :]          # [C, B_SH, teff]
        xt = sl.transpose(2, 1, 0).reshape(teff, RB).astype(bf16)
        # MW[d, (b, c)] = mask[b, c] * W_in[c, d]   (parameter-sized fold)
        m_bc = mask[:, core * B_SH:(core + 1) * B_SH].T         # [B_SH, C]
        mw = (m_bc[:, :, None] * W_in[:, None, :].transpose(1, 0, 2))
        # mw[b, c, d] -> [d, (b, c)]
        mw = np.ascontiguousarray(
            mw.reshape(RB, D_MODEL).T, dtype=np.float32
        )                                                       # [128, 64]


        blob = np.zeros((128, P0 + PW), dtype=bf16)
        kw_c = kw.reshape(nj, 128, D_MODEL).transpose(1, 0, 2)
        xt_c = xt.reshape(nj, 128, RB).transpose(1, 0, 2)
        for j in range(nj):
            blob[:, j * CK:j * CK + D_MODEL] = kw_c[:, j]
            blob[:, j * CK + D_MODEL:(j + 1) * CK] = xt_c[:, j]
        blob[:, P0:P0 + RB] = mw.astype(bf16)
        blob[:, P0 + RB:P0 + RB + 2] = (
            gbias.reshape(-1, 1).view(np.uint16).view(bf16)
        )
        in_maps.append({"blob": blob})

    key = teff
    if key not in _prog_cache:
        _prog_cache[key] = _build_bass(teff)
    nc = _prog_cache[key]

    try:
        results = _get_cached_runner(nc, key)(in_maps)
    except Exception:
        _runner_cache.pop(key, None)
        results = run_bass_kernel_spmd(
            nc, in_maps, core_ids=list(range(N_CORES))
        ).results
    # Readout head on the gathered features: z = Wcombo . yg + blin, then
    # the final sigmoid.  yg is [d, B_SH] per core.
    yg = np.concatenate(
        [results[c]["out"].astype(np.float32) for c in range(N_CORES)], axis=1
    )                                                          # [d, B]
    z = wcombo[:, 0] @ yg + blin_eff
    full = (1.0 / (1.0 + np.exp(-z))).reshape(1, BATCH, 1).astype(np.float32)
    return full
